# revision 22
# baseline (speedup 1.0000x reference)
import os
import numpy as np
import ml_dtypes

# nn_MultiHeadAttention: B=4, S=2048, D=1024, HEADS=16, DIM_HEAD=64.
# Sharding: batch (4) x head-group (2) across 8 cores. Each core computes
# attention for one batch and 8 heads, plus its partial of the output
# projection (row-parallel W0); the two head-group partials per batch are
# summed on the host.
B, S, D = 4, 2048, 1024
HEADS, DH = 16, 64
HPC = 8               # heads per core
E = HPC * DH          # 512 local projection channels
SCALE = DH ** -0.5
P = 128
MT = D // P           # 8 contraction tiles
NPAIR = HPC // 2      # 4 head pairs (= e-chunks of 128)
NTB = S // 512        # 4 query blocks of 512
NJT = S // P          # 16 key tiles of 128
VPW = 65 + 128        # vp columns per pair: h0 [V|1], h1 [1|0*63|V]

_CACHE = {}


def _build():
    if "nc" in _CACHE:
        return _CACHE["nc"]
    import concourse.bacc as bacc
    import concourse.mybir as mybir
    from concourse.tile import TileContext

    f32 = mybir.dt.float32
    bf16 = mybir.dt.bfloat16
    EXP = mybir.ActivationFunctionType.Exp
    MULT = mybir.AluOpType.mult

    nc = bacc.Bacc("TRN2", target_bir_lowering=False, debug=False, num_devices=8)
    qT_d = nc.dram_tensor("qT", [D, S], bf16, kind="ExternalInput")
    kvT_d = nc.dram_tensor("kvT", [D, S], bf16, kind="ExternalInput")
    wq_d = nc.dram_tensor("wqT", [D, E], bf16, kind="ExternalInput")
    wk_d = nc.dram_tensor("wkT", [D, E], bf16, kind="ExternalInput")
    wv_d = nc.dram_tensor("wvT", [D, E], bf16, kind="ExternalInput")
    w0_d = nc.dram_tensor("w0a", [E, D], bf16, kind="ExternalInput")
    out_d = nc.dram_tensor("poutT", [D, S], f32, kind="ExternalOutput")

    with TileContext(nc) as tc:
        with (
            tc.tile_pool(name="pers", bufs=1) as pers,
            tc.tile_pool(name="psS", bufs=1, space="PSUM") as psS,
            tc.tile_pool(name="psO", bufs=1, space="PSUM") as psO,
        ):
            # ---- persistent SBUF tiles (live across phases) ----
            w0a = [pers.tile([P, D], bf16, tag=f"w0{p}", name=f"w0{p}") for p in range(NPAIR)]
            qpt = [pers.tile([P, S], bf16, tag=f"qp{p}", name=f"qp{p}") for p in range(NPAIR)]
            kpt = [pers.tile([P, S], bf16, tag=f"kp{p}", name=f"kp{p}") for p in range(NPAIR)]
            vp = [pers.tile([P, NPAIR * VPW], bf16, tag=f"vp{t}", name=f"vp{t}") for t in range(NJT)]
            onorm = [pers.tile([P, S], bf16, tag=f"on{p}", name=f"on{p}") for p in range(NPAIR)]
            for p in range(NPAIR):
                nc.sync.dma_start(out=w0a[p][:, :], in_=w0_d[p * P:(p + 1) * P, :])

            # psum tag rotation for the projection phase
            ptags = [(psO, "o0"), (psS, "sAB"), (psO, "o1"),
                     (psS, "sAB"), (psS, "sAB")]

            def proj_psum(idx):
                pool, tag = ptags[idx % len(ptags)]
                return pool.tile([P, 512], f32, tag=tag, name=f"proj_{tag}", bufs=(2 if tag == "sAB" else 2))

            # ============ phase A + attention, interleaved per head pair ============
            with tc.tile_pool(name="phA", bufs=1) as pha:
                qTt = [pha.tile([P, S], bf16, tag=f"qT{i}", name=f"qT{i}") for i in range(MT)]
                kvTt = [pha.tile([P, S], bf16, tag=f"kvT{i}", name=f"kvT{i}") for i in range(MT)]
                wqt = [pha.tile([P, E], bf16, tag=f"wq{i}", name=f"wq{i}") for i in range(MT)]
                wkt = [pha.tile([P, E], bf16, tag=f"wk{i}", name=f"wk{i}") for i in range(MT)]
                wvt = [pha.tile([P, E], bf16, tag=f"wv{i}", name=f"wv{i}") for i in range(MT)]

                # DMA in consumption order: KPT ec0 (kv+wk) and QPT ec0 (q+wq)
                # come first, then wv for VP
                for i in range(MT):
                    nc.sync.dma_start(out=kvTt[i][:, :], in_=kvT_d[i * P:(i + 1) * P, :])
                    nc.sync.dma_start(out=wkt[i][:, :], in_=wk_d[i * P:(i + 1) * P, :])
                for i in range(MT):
                    nc.sync.dma_start(out=qTt[i][:, :], in_=qT_d[i * P:(i + 1) * P, :])
                    nc.sync.dma_start(out=wqt[i][:, :], in_=wq_d[i * P:(i + 1) * P, :])
                for i in range(MT):
                    nc.sync.dma_start(out=wvt[i][:, :], in_=wv_d[i * P:(i + 1) * P, :])

                pi = 0
                def vp_block(t):
                    nonlocal pi
                    nc.gpsimd.memset(vp[t][:, :], 0.0)
                    v3 = vp[t].rearrange("x (g c) -> x g c", c=VPW)
                    nc.gpsimd.memset(v3[:, :, 64:66], 1.0)
                    ps = proj_psum(pi)
                    pi += 1
                    for mt in range(MT):
                        nc.tensor.matmul(
                            ps[:, :],
                            lhsT=kvTt[mt][:, t * P:(t + 1) * P],
                            rhs=wvt[mt][:, :],
                            start=(mt == 0), stop=(mt == MT - 1),
                        )
                    p3 = ps.rearrange("x (g c) -> x g c", c=P)
                    nc.vector.tensor_copy(out=v3[:, :, 0:64], in_=p3[:, :, 0:64])
                    nc.vector.tensor_copy(out=v3[:, :, 129:193], in_=p3[:, :, 64:128])

                def proj_block(dst, wt, xt, ec, tb):
                    for tb in [tb]:
                        ps = psS.tile([P, 512], f32, tag="sAB", name="proj_sAB", bufs=2)
                        for mt in range(MT):
                            nc.tensor.matmul(
                                ps[:, :],
                                lhsT=wt[mt][:, ec * P:(ec + 1) * P],
                                rhs=xt[mt][:, tb * 512:(tb + 1) * 512],
                                start=(mt == 0), stop=(mt == MT - 1),
                            )
                        nc.vector.tensor_copy(
                            out=dst[ec][:, tb * 512:(tb + 1) * 512], in_=ps[:, :])

                with (
                    tc.tile_pool(name="at", bufs=3) as atp,
                    tc.tile_pool(name="small", bufs=2) as small,
                ):
                    for t in range(NJT):
                        vp_block(t)
                    for tb in range(NTB):
                        proj_block(kpt, wkt, kvTt, 0, tb)
                    for tb in range(NTB):
                        proj_block(qpt, wqt, qTt, 0, tb)
                    for p in range(NPAIR):
                        if p > 0:
                            for tb in range(NTB):
                                proj_block(kpt, wkt, kvTt, p, tb)
                            for tb in range(NTB):
                                proj_block(qpt, wqt, qTt, p, tb)
                        q0 = qpt[p]
                        vslc0 = (p * VPW, p * VPW + 65)
                        vslc1 = (p * VPW + 65, (p + 1) * VPW)
                        for ibl in range(4):
                            po0 = psO.tile([65, 512], f32, tag="o0", name="po0", bufs=2)
                            po1 = psO.tile([P, 512], f32, tag="o1", name="po1", bufs=2)
                            for jg in range(NJT // 2):
                                js = (2 * jg, 2 * jg + 1)
                                sabs, ats = [], []
                                for j in js:
                                    sAB = psS.tile([P, 1024], f32, tag="sAB", name="sAB", bufs=2)
                                    sabs.append(sAB)
                                    nc.tensor.matmul(
                                        sAB[:, 0:512],
                                        lhsT=kpt[p][0:64, j * P:(j + 1) * P],
                                        rhs=q0[0:64, ibl * 512:(ibl + 1) * 512],
                                        start=True, stop=True,
                                        tile_position=(0, 0),
                                    )
                                    nc.tensor.matmul(
                                        sAB[:, 512:1024],
                                        lhsT=kpt[p][64:128, j * P:(j + 1) * P],
                                        rhs=q0[64:128, ibl * 512:(ibl + 1) * 512],
                                        start=True, stop=True,
                                        tile_position=(64, 0),
                                    )
                                for sAB in sabs:
                                    at = atp.tile([P, 1024], bf16, tag="at", name="at")
                                    ats.append(at)
                                    nc.scalar.activation(at[:, :], sAB[:, :], EXP, scale=SCALE)
                                for j, at in zip(js, ats):
                                    nc.tensor.matmul(
                                        po0[:, :],
                                        lhsT=vp[j][:, vslc0[0]:vslc0[1]],
                                        rhs=at[:, 0:512],
                                        start=(j == 0), stop=(j == NJT - 1),
                                    )
                                    nc.tensor.matmul(
                                        po1[:, :],
                                        lhsT=vp[j][:, vslc1[0]:vslc1[1]],
                                        rhs=at[:, 512:1024],
                                        start=(j == 0), stop=(j == NJT - 1),
                                    )
                            # normalize: onorm[e, i] = po[e, i] / sums[i]
                            srow0 = small.tile([1, 512], f32, tag="srow0", name="srow0")
                            srow1 = small.tile([1, 512], f32, tag="srow1", name="srow1")
                            nc.vector.tensor_copy(out=srow0[:, :], in_=po0[64:65, :])
                            nc.vector.tensor_copy(out=srow1[:, :], in_=po1[0:1, :])
                            rrow0 = small.tile([1, 512], f32, tag="rrow0", name="rrow0")
                            rrow1 = small.tile([1, 512], f32, tag="rrow1", name="rrow1")
                            nc.vector.reciprocal_approx_fast(out=rrow0[:, :], in_=srow0[:, :])
                            nc.vector.reciprocal_approx_fast(out=rrow1[:, :], in_=srow1[:, :])
                            rbs = small.tile([P, 512], f32, tag="rbs", name="rbs")
                            rbt = small.tile([64, 512], f32, tag="rbt", name="rbt")
                            nc.gpsimd.partition_broadcast(rbs[0:64, :], rrow0[0:1, :], channels=64)
                            nc.gpsimd.partition_broadcast(rbt[0:64, :], rrow1[0:1, :], channels=64)
                            nc.sync.dma_start(out=rbs[64:128, :], in_=rbt[0:64, :])
                            nc.vector.tensor_tensor(
                                out=onorm[p][0:64, ibl * 512:(ibl + 1) * 512],
                                in0=po0[0:64, :], in1=rbs[0:64, :], op=MULT)
                            nc.vector.tensor_tensor(
                                out=onorm[p][64:128, ibl * 512:(ibl + 1) * 512],
                                in0=po1[64:128, :], in1=rbs[64:128, :], op=MULT)

            # ================= output projection =================
            with (
                tc.tile_pool(name="ob", bufs=3) as obp,
            ):
                # output projection: poutT[Dc, t] = sum_e W0a[e, Dc] onorm[e, t]
                fi = 0
                for tb in range(NTB):
                    for dc in range(D // P):
                        pp = psO.tile([P, 512], f32,
                                      tag=("o0" if fi % 2 == 0 else "o1"), name="pp", bufs=2)
                        fi += 1
                        for p in range(NPAIR):
                            nc.tensor.matmul(
                                pp[:, :],
                                lhsT=w0a[p][:, dc * P:(dc + 1) * P],
                                rhs=onorm[p][:, tb * 512:(tb + 1) * 512],
                                start=(p == 0), stop=(p == NPAIR - 1),
                            )
                        ob = obp.tile([P, 512], f32, tag="ob", name="ob")
                        nc.vector.tensor_copy(out=ob[:, :], in_=pp[:, :])
                        nc.sync.dma_start(
                            out=out_d[dc * P:(dc + 1) * P, tb * 512:(tb + 1) * 512],
                            in_=ob[:, :])

    nc.compile()
    _CACHE["nc"] = nc
    return nc


def _prep_weights(Wq, Wkv, W0):
    bf = ml_dtypes.bfloat16
    per_group = {}
    for g in range(2):
        hg = np.arange(HPC) + g * HPC            # global head ids
        d = np.arange(DH)
        # e_local = h_l*64 + d ; reference maps: e_q = d*16+h, e_k = d*32+h,
        # e_v = d*32+16+h, out channel = h*64+d
        idx_q = (d[None, :] * HEADS + hg[:, None]).reshape(-1)
        idx_k = (d[None, :] * 2 * HEADS + hg[:, None]).reshape(-1)
        idx_v = (d[None, :] * 2 * HEADS + HEADS + hg[:, None]).reshape(-1)
        idx_o = (hg[:, None] * DH + d[None, :]).reshape(-1)
        per_group[g] = {
            "wqT": np.ascontiguousarray(Wq[idx_q, :].T).astype(bf),
            "wkT": np.ascontiguousarray(Wkv[idx_k, :].T).astype(bf),
            "wvT": np.ascontiguousarray(Wkv[idx_v, :].T).astype(bf),
            "w0a": np.ascontiguousarray(W0[:, idx_o].T).astype(bf),
        }
    return per_group


def kernel(q, kv, Wq, Wkv, W0):
    from concourse.bass_utils import run_bass_kernel_spmd

    q = np.asarray(q, dtype=np.float32)
    kv = np.asarray(kv, dtype=np.float32)
    Wq = np.asarray(Wq, dtype=np.float32)
    Wkv = np.asarray(Wkv, dtype=np.float32)
    W0 = np.asarray(W0, dtype=np.float32)

    nc = _build()
    bf = ml_dtypes.bfloat16
    wg = _prep_weights(Wq, Wkv, W0)
    in_maps = []
    for c in range(8):
        b, g = divmod(c, 2)
        in_maps.append({
            "qT": np.ascontiguousarray(q[b].T).astype(bf),
            "kvT": np.ascontiguousarray(kv[b].T).astype(bf),
            "wqT": wg[g]["wqT"],
            "wkT": wg[g]["wkT"],
            "wvT": wg[g]["wvT"],
            "w0a": wg[g]["w0a"],
        })
    trace = bool(int(os.environ.get("KERNEL_TRACE", "0")))
    res = run_bass_kernel_spmd(nc, in_maps, list(range(8)), trace=trace)
    _CACHE["last_result"] = res
    out = np.empty((B, S, D), dtype=np.float32)
    for b in range(B):
        acc = res.results[2 * b]["poutT"] + res.results[2 * b + 1]["poutT"]
        out[b] = acc.T
    return out


# revision 23
# speedup vs baseline: 1.0733x; 1.0733x over previous
import os
import numpy as np
import ml_dtypes

# nn_MultiHeadAttention: B=4, S=2048, D=1024, HEADS=16, DIM_HEAD=64.
# Sharding: batch (4) x head-group (2) across 8 cores. Each core computes
# attention for one batch and 8 heads, plus its partial of the output
# projection (row-parallel W0); the two head-group partials per batch are
# summed on the host.
B, S, D = 4, 2048, 1024
HEADS, DH = 16, 64
HPC = 8               # heads per core
E = HPC * DH          # 512 local projection channels
SCALE = DH ** -0.5
P = 128
MT = D // P           # 8 contraction tiles
NPAIR = HPC // 2      # 4 head pairs (= e-chunks of 128)
NTB = S // 512        # 4 query blocks of 512
NJT = S // P          # 16 key tiles of 128
VPW = 65 + 128        # vp columns per pair: h0 [V|1], h1 [1|0*63|V]

_CACHE = {}


def _build():
    if "nc" in _CACHE:
        return _CACHE["nc"]
    import concourse.bacc as bacc
    import concourse.mybir as mybir
    from concourse.tile import TileContext

    f32 = mybir.dt.float32
    bf16 = mybir.dt.bfloat16
    EXP = mybir.ActivationFunctionType.Exp
    MULT = mybir.AluOpType.mult

    nc = bacc.Bacc("TRN2", target_bir_lowering=False, debug=False, num_devices=8)
    qT_d = nc.dram_tensor("qT", [D, S], bf16, kind="ExternalInput")
    kvT_d = nc.dram_tensor("kvT", [D, S], bf16, kind="ExternalInput")
    wq_d = nc.dram_tensor("wqT", [D, E], bf16, kind="ExternalInput")
    wk_d = nc.dram_tensor("wkT", [D, E], bf16, kind="ExternalInput")
    wv_d = nc.dram_tensor("wvT", [D, E], bf16, kind="ExternalInput")
    w0_d = nc.dram_tensor("w0a", [E, D], bf16, kind="ExternalInput")
    out_d = nc.dram_tensor("poutT", [D, S], f32, kind="ExternalOutput")

    with TileContext(nc) as tc:
        with (
            tc.tile_pool(name="pers", bufs=1) as pers,
            tc.tile_pool(name="psS", bufs=1, space="PSUM") as psS,
            tc.tile_pool(name="psO", bufs=1, space="PSUM") as psO,
        ):
            # ---- persistent SBUF tiles (live across phases) ----
            w0a = [pers.tile([P, D], bf16, tag=f"w0{p}", name=f"w0{p}") for p in range(NPAIR)]
            qpt = [pers.tile([P, S], bf16, tag=f"qp{p}", name=f"qp{p}") for p in range(NPAIR)]
            kpt = [pers.tile([P, S], bf16, tag=f"kp{p}", name=f"kp{p}") for p in range(NPAIR)]
            vp = [pers.tile([P, NPAIR * VPW], bf16, tag=f"vp{t}", name=f"vp{t}") for t in range(NJT)]
            onorm = [pers.tile([P, S], bf16, tag=f"on{p}", name=f"on{p}") for p in range(NPAIR)]
            for p in range(NPAIR):
                nc.sync.dma_start(out=w0a[p][:, :], in_=w0_d[p * P:(p + 1) * P, :])

            # psum tag rotation for the projection phase
            ptags = [(psO, "o0"), (psS, "sAB"), (psO, "o1"),
                     (psS, "sAB"), (psS, "sAB")]

            def proj_psum(idx):
                pool, tag = ptags[idx % len(ptags)]
                return pool.tile([P, 512], f32, tag=tag, name=f"proj_{tag}", bufs=(2 if tag == "sAB" else 2))

            # ============ phase A + attention, interleaved per head pair ============
            with tc.tile_pool(name="phA", bufs=1) as pha:
                qTt = [pha.tile([P, S], bf16, tag=f"qT{i}", name=f"qT{i}") for i in range(MT)]
                kvTt = [pha.tile([P, S], bf16, tag=f"kvT{i}", name=f"kvT{i}") for i in range(MT)]
                wqt = [pha.tile([P, E], bf16, tag=f"wq{i}", name=f"wq{i}") for i in range(MT)]
                wkt = [pha.tile([P, E], bf16, tag=f"wk{i}", name=f"wk{i}") for i in range(MT)]
                wvt = [pha.tile([P, E], bf16, tag=f"wv{i}", name=f"wv{i}") for i in range(MT)]

                # DMA in consumption order: KPT ec0 (kv+wk) and QPT ec0 (q+wq)
                # come first, then wv for VP
                for i in range(MT):
                    nc.sync.dma_start(out=wvt[i][:, :], in_=wv_d[i * P:(i + 1) * P, :])
                    nc.sync.dma_start(out=kvTt[i][:, :], in_=kvT_d[i * P:(i + 1) * P, :])
                for i in range(MT):
                    nc.sync.dma_start(out=wkt[i][:, :], in_=wk_d[i * P:(i + 1) * P, :])
                for i in range(MT):
                    nc.sync.dma_start(out=wqt[i][:, :], in_=wq_d[i * P:(i + 1) * P, :])
                    nc.sync.dma_start(out=qTt[i][:, :], in_=qT_d[i * P:(i + 1) * P, :])

                pi = 0
                def vp_block(t):
                    nonlocal pi
                    nc.gpsimd.memset(vp[t][:, :], 0.0)
                    v3 = vp[t].rearrange("x (g c) -> x g c", c=VPW)
                    nc.gpsimd.memset(v3[:, :, 64:66], 1.0)
                    ps = proj_psum(pi)
                    pi += 1
                    for mt in range(MT):
                        nc.tensor.matmul(
                            ps[:, :],
                            lhsT=kvTt[mt][:, t * P:(t + 1) * P],
                            rhs=wvt[mt][:, :],
                            start=(mt == 0), stop=(mt == MT - 1),
                        )
                    p3 = ps.rearrange("x (g c) -> x g c", c=P)
                    nc.vector.tensor_copy(out=v3[:, :, 0:64], in_=p3[:, :, 0:64])
                    nc.vector.tensor_copy(out=v3[:, :, 129:193], in_=p3[:, :, 64:128])

                def proj_block(dst, wt, xt, ec, tb):
                    nonlocal pi
                    for tb in [tb]:
                        ps = proj_psum(pi)
                        pi += 1
                        for mt in range(MT):
                            nc.tensor.matmul(
                                ps[:, :],
                                lhsT=wt[mt][:, ec * P:(ec + 1) * P],
                                rhs=xt[mt][:, tb * 512:(tb + 1) * 512],
                                start=(mt == 0), stop=(mt == MT - 1),
                            )
                        nc.vector.tensor_copy(
                            out=dst[ec][:, tb * 512:(tb + 1) * 512], in_=ps[:, :])

                with (
                    tc.tile_pool(name="at", bufs=3) as atp,
                    tc.tile_pool(name="small", bufs=2) as small,
                ):
                    for t in range(NJT):
                        vp_block(t)
                    for tb in range(NTB):
                        proj_block(kpt, wkt, kvTt, 0, tb)
                    for tb in range(NTB):
                        proj_block(qpt, wqt, qTt, 0, tb)
                    for p in range(NPAIR):
                        if p > 0:
                            for tb in range(NTB):
                                proj_block(kpt, wkt, kvTt, p, tb)
                            for tb in range(NTB):
                                proj_block(qpt, wqt, qTt, p, tb)
                        q0 = qpt[p]
                        vslc0 = (p * VPW, p * VPW + 65)
                        vslc1 = (p * VPW + 65, (p + 1) * VPW)
                        for ibl in range(4):
                            po0 = psO.tile([65, 512], f32, tag="o0", name="po0", bufs=2)
                            po1 = psO.tile([P, 512], f32, tag="o1", name="po1", bufs=2)
                            for jg in range(NJT // 2):
                                js = (2 * jg, 2 * jg + 1)
                                sabs, ats = [], []
                                for j in js:
                                    sAB = psS.tile([P, 1024], f32, tag="sAB", name="sAB", bufs=2)
                                    sabs.append(sAB)
                                    nc.tensor.matmul(
                                        sAB[:, 0:512],
                                        lhsT=kpt[p][0:64, j * P:(j + 1) * P],
                                        rhs=q0[0:64, ibl * 512:(ibl + 1) * 512],
                                        start=True, stop=True,
                                        tile_position=(0, 0),
                                    )
                                    nc.tensor.matmul(
                                        sAB[:, 512:1024],
                                        lhsT=kpt[p][64:128, j * P:(j + 1) * P],
                                        rhs=q0[64:128, ibl * 512:(ibl + 1) * 512],
                                        start=True, stop=True,
                                        tile_position=(64, 0),
                                    )
                                for sAB in sabs:
                                    at = atp.tile([P, 1024], bf16, tag="at", name="at")
                                    ats.append(at)
                                    nc.scalar.activation(at[:, :], sAB[:, :], EXP, scale=SCALE)
                                for j, at in zip(js, ats):
                                    nc.tensor.matmul(
                                        po0[:, :],
                                        lhsT=vp[j][:, vslc0[0]:vslc0[1]],
                                        rhs=at[:, 0:512],
                                        start=(j == 0), stop=(j == NJT - 1),
                                    )
                                    nc.tensor.matmul(
                                        po1[:, :],
                                        lhsT=vp[j][:, vslc1[0]:vslc1[1]],
                                        rhs=at[:, 512:1024],
                                        start=(j == 0), stop=(j == NJT - 1),
                                    )
                            # normalize: onorm[e, i] = po[e, i] / sums[i]
                            srow0 = small.tile([1, 512], f32, tag="srow0", name="srow0")
                            srow1 = small.tile([1, 512], f32, tag="srow1", name="srow1")
                            nc.vector.tensor_copy(out=srow0[:, :], in_=po0[64:65, :])
                            nc.vector.tensor_copy(out=srow1[:, :], in_=po1[0:1, :])
                            rrow0 = small.tile([1, 512], f32, tag="rrow0", name="rrow0")
                            rrow1 = small.tile([1, 512], f32, tag="rrow1", name="rrow1")
                            nc.vector.reciprocal_approx_fast(out=rrow0[:, :], in_=srow0[:, :])
                            nc.vector.reciprocal_approx_fast(out=rrow1[:, :], in_=srow1[:, :])
                            rbs = small.tile([P, 512], f32, tag="rbs", name="rbs")
                            rbt = small.tile([64, 512], f32, tag="rbt", name="rbt")
                            nc.gpsimd.partition_broadcast(rbs[0:64, :], rrow0[0:1, :], channels=64)
                            nc.gpsimd.partition_broadcast(rbt[0:64, :], rrow1[0:1, :], channels=64)
                            nc.sync.dma_start(out=rbs[64:128, :], in_=rbt[0:64, :])
                            nc.vector.tensor_tensor(
                                out=onorm[p][0:64, ibl * 512:(ibl + 1) * 512],
                                in0=po0[0:64, :], in1=rbs[0:64, :], op=MULT)
                            nc.vector.tensor_tensor(
                                out=onorm[p][64:128, ibl * 512:(ibl + 1) * 512],
                                in0=po1[64:128, :], in1=rbs[64:128, :], op=MULT)

            # ================= output projection =================
            with (
                tc.tile_pool(name="ob", bufs=3) as obp,
            ):
                # output projection: poutT[Dc, t] = sum_e W0a[e, Dc] onorm[e, t]
                fi = 0
                for tb in range(NTB):
                    for dc in range(D // P):
                        pp = psO.tile([P, 512], f32,
                                      tag=("o0" if fi % 2 == 0 else "o1"), name="pp", bufs=2)
                        fi += 1
                        for p in range(NPAIR):
                            nc.tensor.matmul(
                                pp[:, :],
                                lhsT=w0a[p][:, dc * P:(dc + 1) * P],
                                rhs=onorm[p][:, tb * 512:(tb + 1) * 512],
                                start=(p == 0), stop=(p == NPAIR - 1),
                            )
                        ob = obp.tile([P, 512], f32, tag="ob", name="ob")
                        nc.vector.tensor_copy(out=ob[:, :], in_=pp[:, :])
                        nc.sync.dma_start(
                            out=out_d[dc * P:(dc + 1) * P, tb * 512:(tb + 1) * 512],
                            in_=ob[:, :])

    nc.compile()
    _CACHE["nc"] = nc
    return nc


def _prep_weights(Wq, Wkv, W0):
    bf = ml_dtypes.bfloat16
    per_group = {}
    for g in range(2):
        hg = np.arange(HPC) + g * HPC            # global head ids
        d = np.arange(DH)
        # e_local = h_l*64 + d ; reference maps: e_q = d*16+h, e_k = d*32+h,
        # e_v = d*32+16+h, out channel = h*64+d
        idx_q = (d[None, :] * HEADS + hg[:, None]).reshape(-1)
        idx_k = (d[None, :] * 2 * HEADS + hg[:, None]).reshape(-1)
        idx_v = (d[None, :] * 2 * HEADS + HEADS + hg[:, None]).reshape(-1)
        idx_o = (hg[:, None] * DH + d[None, :]).reshape(-1)
        per_group[g] = {
            "wqT": np.ascontiguousarray(Wq[idx_q, :].T).astype(bf),
            "wkT": np.ascontiguousarray(Wkv[idx_k, :].T).astype(bf),
            "wvT": np.ascontiguousarray(Wkv[idx_v, :].T).astype(bf),
            "w0a": np.ascontiguousarray(W0[:, idx_o].T).astype(bf),
        }
    return per_group


def kernel(q, kv, Wq, Wkv, W0):
    from concourse.bass_utils import run_bass_kernel_spmd

    q = np.asarray(q, dtype=np.float32)
    kv = np.asarray(kv, dtype=np.float32)
    Wq = np.asarray(Wq, dtype=np.float32)
    Wkv = np.asarray(Wkv, dtype=np.float32)
    W0 = np.asarray(W0, dtype=np.float32)

    nc = _build()
    bf = ml_dtypes.bfloat16
    wg = _prep_weights(Wq, Wkv, W0)
    in_maps = []
    for c in range(8):
        b, g = divmod(c, 2)
        in_maps.append({
            "qT": np.ascontiguousarray(q[b].T).astype(bf),
            "kvT": np.ascontiguousarray(kv[b].T).astype(bf),
            "wqT": wg[g]["wqT"],
            "wkT": wg[g]["wkT"],
            "wvT": wg[g]["wvT"],
            "w0a": wg[g]["w0a"],
        })
    trace = bool(int(os.environ.get("KERNEL_TRACE", "0")))
    res = run_bass_kernel_spmd(nc, in_maps, list(range(8)), trace=trace)
    _CACHE["last_result"] = res
    out = np.empty((B, S, D), dtype=np.float32)
    for b in range(B):
        acc = res.results[2 * b]["poutT"] + res.results[2 * b + 1]["poutT"]
        out[b] = acc.T
    return out
